# revision 1
# baseline (speedup 1.0000x reference)
"""LocalFusionModule kernel for 8 trn2 NeuronCores.

Strategy (per sharding hint): data-parallel over batch b — each of the 8
cores handles one batch element. The heavy work (ref-normalization, the
[num x HW] cosine-similarity matmul against every reference position, and
the top-1 argmax over HW) runs on-device via a pmap'd XLA program, one
batch element per core. Only the tiny ref_idx [n, num] result returns to
host; the cheap column gathers / weighted fusion / scatter run host-side.

Falls back to a pure-numpy implementation if the device path is
unavailable. Output matches reference(): (out, feat_indices, ref_idx).
"""

import numpy as np

B, NTOT, C, H, W = 8, 8, 128, 64, 64
HW = H * W
NUM = 1024
EPS = 1e-12

_PMAP_CACHE = {}


def _get_pmapped(n):
    """Build/cache the pmap'd device function for n kept refs."""
    if n in _PMAP_CACHE:
        return _PMAP_CACHE[n]
    import jax
    import jax.numpy as jnp

    def dev_fn(feat_flat, refs_flat, fidx):
        # feat_flat [C,HW], refs_flat [n,C,HW], fidx [num] -> ref_idx [n,num]
        # Channel-wise L2-normalize each reference position. The query side
        # needs no normalization: a positive per-query scale cannot change
        # the argmax over positions k.
        n2 = jnp.sum(refs_flat * refs_flat, axis=1, keepdims=True)
        w_refs = refs_flat / jnp.maximum(jnp.sqrt(n2), EPS)
        fsel = jnp.take(feat_flat, fidx, axis=1)              # [C,num]
        fx = jnp.einsum("cq,nck->nqk", fsel, w_refs)          # [n,num,HW]
        return jnp.argmax(fx, axis=-1).astype(jnp.int32)

    fn = jax.pmap(dev_fn)
    _PMAP_CACHE[n] = fn
    return fn


def _ref_idx_numpy(feat_flat, refs_flat, fidx):
    """Host fallback: per-(b,n) argmax of cosine similarity."""
    b, n = refs_flat.shape[0], refs_flat.shape[1]
    out = np.empty((b, n, fidx.shape[1]), np.int32)
    for bi in range(b):
        fsel = feat_flat[bi][:, fidx[bi]]                     # [C,num]
        for ni in range(n):
            r = refs_flat[bi, ni]                             # [C,HW]
            w = r / np.maximum(np.sqrt((r * r).sum(0, keepdims=True)), EPS)
            fx = fsel.T @ w                                   # [num,HW]
            out[bi, ni] = fx.argmax(-1).astype(np.int32)
    return out


def kernel(feat, refs, similarity, feat_indices, index):
    feat = np.asarray(feat, np.float32)
    refs = np.asarray(refs, np.float32)
    similarity = np.asarray(similarity, np.float32)
    feat_indices = np.asarray(feat_indices, np.int32)
    idx = int(index)

    b, ntot = refs.shape[0], refs.shape[1]
    c, hw = refs.shape[2], refs.shape[3] * refs.shape[4]
    keep = [i for i in range(ntot) if i != idx]
    n = len(keep)

    feat_flat = feat.reshape(b, c, hw)
    refs_flat = np.ascontiguousarray(refs[:, keep]).reshape(b, n, c, hw)
    base_sim = similarity[:, idx]                             # [b]
    ref_sims = similarity[:, keep]                            # [b,n]

    try:
        fn = _get_pmapped(n)
        ref_idx = np.asarray(fn(feat_flat, refs_flat, feat_indices))
    except Exception:
        ref_idx = _ref_idx_numpy(feat_flat, refs_flat, feat_indices)
    ref_idx = ref_idx.astype(np.int32)

    # Host-side: gather winning ref columns, similarity-weighted fusion,
    # scatter back into the feature map.
    out = feat_flat.copy()
    for bi in range(b):
        fidx = feat_indices[bi]
        fused = base_sim[bi] * feat_flat[bi][:, fidx]         # [c,num]
        for ni in range(n):
            fused += ref_sims[bi, ni] * refs_flat[bi, ni][:, ref_idx[bi, ni]]
        out[bi][:, fidx] = fused

    return out.reshape(b, c, H, W), feat_indices, ref_idx


# revision 3
# speedup vs baseline: 1.1004x; 1.1004x over previous
"""LocalFusionModule kernel for 8 trn2 NeuronCores.

Strategy (per sharding hint): data-parallel over batch b — each of the 8
cores handles one batch element. The heavy work (ref-normalization, the
[num x HW] cosine-similarity matmul against every reference position, and
the top-1 argmax over HW) runs on-device via a pmap'd XLA program, one
batch element per core. Only the tiny ref_idx [n, num] result returns to
host; the cheap column gathers / weighted fusion / scatter run host-side.

Falls back to a pure-numpy implementation if the device path is
unavailable. Output matches reference(): (out, feat_indices, ref_idx).
"""

import numpy as np

B, NTOT, C, H, W = 8, 8, 128, 64, 64
HW = H * W
NUM = 1024
EPS = 1e-12

_PMAP_CACHE = {}


def _get_pmapped(n, fused):
    """Build/cache the pmap'd device function for n kept refs."""
    key = (n, fused)
    if key in _PMAP_CACHE:
        return _PMAP_CACHE[key]
    import jax
    import jax.numpy as jnp

    def dev_fn(feat_flat, refs_flat, fidx):
        # feat_flat [C,HW], refs_flat [n,C,HW], fidx [num] -> ref_idx [n,num]
        # Channel-wise L2-normalize each reference position. The query side
        # needs no normalization: a positive per-query scale cannot change
        # the argmax over positions k.
        n2 = jnp.sum(refs_flat * refs_flat, axis=1, keepdims=True)
        w_refs = refs_flat / jnp.maximum(jnp.sqrt(n2), EPS)
        fsel = jnp.take(feat_flat, fidx, axis=1)              # [C,num]
        fx = jnp.einsum("cq,nck->nqk", fsel, w_refs)          # [n,num,HW]
        return jnp.argmax(fx, axis=-1).astype(jnp.int32)

    def dev_fn2(feat_flat, refs_flat, fidx, base_sim, ref_sims):
        # Also gather the winning columns and do the similarity-weighted
        # fusion on-device; host only scatters the [C,num] result.
        n2 = jnp.sum(refs_flat * refs_flat, axis=1, keepdims=True)
        w_refs = refs_flat / jnp.maximum(jnp.sqrt(n2), EPS)
        fsel = jnp.take(feat_flat, fidx, axis=1)
        fx = jnp.einsum("cq,nck->nqk", fsel, w_refs)
        ridx = jnp.argmax(fx, axis=-1).astype(jnp.int32)
        rsel = jnp.take_along_axis(refs_flat, ridx[:, None, :], axis=2)
        out = base_sim * fsel + jnp.einsum("n,ncq->cq", ref_sims, rsel)
        return ridx, out

    fn = jax.pmap(dev_fn2 if fused else dev_fn)
    _PMAP_CACHE[key] = fn
    return fn


def _ref_idx_numpy(feat_flat, refs_flat, fidx):
    """Host fallback: per-(b,n) argmax of cosine similarity."""
    b, n = refs_flat.shape[0], refs_flat.shape[1]
    out = np.empty((b, n, fidx.shape[1]), np.int32)
    for bi in range(b):
        fsel = feat_flat[bi][:, fidx[bi]]                     # [C,num]
        for ni in range(n):
            r = refs_flat[bi, ni]                             # [C,HW]
            w = r / np.maximum(np.sqrt((r * r).sum(0, keepdims=True)), EPS)
            fx = fsel.T @ w                                   # [num,HW]
            out[bi, ni] = fx.argmax(-1).astype(np.int32)
    return out


def kernel(feat, refs, similarity, feat_indices, index):
    feat = np.asarray(feat, np.float32)
    refs = np.asarray(refs, np.float32)
    similarity = np.asarray(similarity, np.float32)
    feat_indices = np.asarray(feat_indices, np.int32)
    idx = int(index)

    b, ntot = refs.shape[0], refs.shape[1]
    c, hw = refs.shape[2], refs.shape[3] * refs.shape[4]
    keep = [i for i in range(ntot) if i != idx]
    n = len(keep)

    feat_flat = feat.reshape(b, c, hw)
    refs_flat = np.ascontiguousarray(refs[:, keep]).reshape(b, n, c, hw)
    base_sim = similarity[:, idx]                             # [b]
    ref_sims = similarity[:, keep]                            # [b,n]

    ref_idx = fused_dev = None
    try:
        fn = _get_pmapped(n, fused=True)
        ridx, fused_dev = fn(feat_flat, refs_flat, feat_indices, base_sim, ref_sims)
        ref_idx, fused_dev = np.asarray(ridx), np.asarray(fused_dev)
    except Exception:
        try:
            fn = _get_pmapped(n, fused=False)
            ref_idx = np.asarray(fn(feat_flat, refs_flat, feat_indices))
        except Exception:
            ref_idx = _ref_idx_numpy(feat_flat, refs_flat, feat_indices)
    ref_idx = ref_idx.astype(np.int32)

    # Host-side: scatter fused columns back into the feature map (and, on
    # fallback paths, compute the fusion itself).
    out = feat_flat.copy()
    for bi in range(b):
        fidx = feat_indices[bi]
        if fused_dev is not None:
            fused = fused_dev[bi]
        else:
            fused = base_sim[bi] * feat_flat[bi][:, fidx]     # [c,num]
            for ni in range(n):
                fused += ref_sims[bi, ni] * refs_flat[bi, ni][:, ref_idx[bi, ni]]
        out[bi][:, fidx] = fused

    return out.reshape(b, c, H, W), feat_indices, ref_idx
